# revision 1
# baseline (speedup 1.0000x reference)
"""Trainium2 Bass kernel for the Kalman graphical-model message-passing problem.

reference math (B=64, D=8, M=4, S=50000):
    m1 = -Qinv @ (xs - F @ x_past)            (B, D, S)
    m2 = FtQinv @ (x_fut - F @ xs)            (B, D, S)
    m3 = HtRinv @ ys_t - (HtRinv @ H) @ xs    (B, D, S)
with x_past/x_fut edge-replicated 1-sample shifts of xs along S.

Reformulated as pure (tiny matrix) x (data) products with host-precomputed
weights:
    m1 = A1 @ xs + B1 @ x_past        A1 = -Qinv,        B1 = Qinv @ F
    m2 = A2 @ xs + B2 @ x_fut         A2 = -F'QinvF,     B2 = F' @ Qinv
    m3 = A3 @ xs + sum_m C3[:, m] * ys[:, :, m]
                                      A3 = -(C3 @ H),    C3 = H' @ Rinv

Device layout (per core: 8 batches, data-parallel across 8 cores):
  A supertile covers 16 consecutive 1024-sample groups of one batch.  SBUF X
  tile [128, 1026]: partition 8g+j = (group g, state j), columns = samples
  with 1 halo column each side, so cur/past/fut are just column offsets 1/0/2
  of the same tile.  Weights are 16x block-diagonal [128, 128] lhsT matrices
  -> each output tile is a PSUM-accumulated chain of float32r matmuls at full
  column rate, processed in two 512-column halves (PSUM bank limit).  ys is
  loaded contiguously as [16, 1024*4] (partition = group, free = (t, m)
  interleaved) and contracted over m with 4 accumulating matmuls whose rhs
  access patterns stride by 4 elements.  The three outputs live in one
  [bc, D, 3, s] DRAM tensor so each supertile's store is a single DMA whose
  (state, output) dims merge into one stride-s run of 24.
"""

import os
from contextlib import ExitStack

import numpy as np

import concourse.bacc as bacc
import concourse.bass as bass
import concourse.mybir as mybir
import concourse.tile as tile
from concourse.bass_utils import run_bass_kernel_spmd

F32 = mybir.dt.float32
F32R = mybir.dt.float32r

B, D, M, S = 64, 8, 4, 50000
N_CORES = 8
BC = B // N_CORES  # batches per core
NG = 16            # sample groups packed into the 128 partitions
TCG = 1024         # samples per group per supertile
MW = 512           # matmul free-dim / PSUM bank width


def _build_nc(bc=BC, s=S):
    variant = os.environ.get("KERNEL_VARIANT", "full")  # perf bisection only
    super_sz = NG * TCG
    n_full = s // super_sz
    rem = s - n_full * super_sz
    # fp32r matmuls need an even free-dim count: round the tail width up to
    # even and overlap the previous supertile (overlapped samples are computed
    # twice with identical results).
    tc_tail = -(-rem // NG)
    tc_tail += tc_tail % 2
    tail_base = s - NG * tc_tail
    assert rem > 0 and 2 <= tc_tail <= MW and tail_base >= 1, (s, rem, tc_tail)

    nc = bacc.Bacc(trn_type="TRN2")
    xs = nc.dram_tensor("xs", [bc, D, s], F32R, kind="ExternalInput")
    ys = nc.dram_tensor("ys", [bc, s, M], F32R, kind="ExternalInput")
    w = nc.dram_tensor("w_all", [128, 9 * 128], F32R, kind="ExternalInput")
    # [b, j, o, s] layout: the store's (j, o) dims merge into one stride-s
    # run of 24, keeping the DMA access pattern at 3 dims.
    m_all = nc.dram_tensor("m_all", [bc, D, 3, s], F32, kind="ExternalOutput")

    with tile.TileContext(nc) as tc, ExitStack() as ctx:
        singles = ctx.enter_context(tc.tile_pool(name="singles", bufs=1))
        xp = ctx.enter_context(tc.tile_pool(name="xp", bufs=3))
        yp = ctx.enter_context(tc.tile_pool(name="yp", bufs=3))
        op = ctx.enter_context(tc.tile_pool(name="op", bufs=3))
        pp = ctx.enter_context(tc.tile_pool(name="pp", bufs=2, space="PSUM"))

        w_sb = singles.tile([128, 9 * 128], F32R, tag="w")
        nc.sync.dma_start(out=w_sb[:], in_=w[:, :])
        wr = w_sb[:]

        for b in range(bc):
            xoff = b * D * s
            yoff = b * s * M
            ooff = b * D * 3 * s
            for k in range(n_full + 1):
                is_tail = k == n_full
                tcw = tc_tail if is_tail else TCG
                base = tail_base if is_tail else k * super_sz
                cols = tcw + 2

                # --- load xs supertile with halo columns -------------------
                x_t = xp.tile([128, TCG + 2], F32R, tag="x")
                if k == 0:
                    # columns 1..cols-1 hold samples tcw*g .. tcw*g+tcw
                    nc.sync.dma_start(
                        out=x_t[:, 1:cols],
                        in_=bass.AP(xs, xoff, [[tcw, NG], [s, D], [1, cols - 1]]),
                    )
                    # group 0: replicate sample 0 into the past halo (aligned
                    # 32-partition copy; the halo DMA below overwrites 8..32)
                    nc.vector.tensor_copy(out=x_t[0:32, 0:1], in_=x_t[0:32, 1:2])
                    # past-halo col 0 for groups 1..15 = sample tcw*g - 1
                    nc.sync.dma_start(
                        out=x_t[D:128, 0:1],
                        in_=bass.AP(
                            xs, xoff + tcw - 1, [[tcw, NG - 1], [s, D], [1, 1]]
                        ),
                    )
                elif not is_tail:
                    nc.sync.dma_start(
                        out=x_t[:, 0:cols],
                        in_=bass.AP(
                            xs, xoff + base - 1, [[tcw, NG], [s, D], [1, cols]]
                        ),
                    )
                else:
                    # tail: columns 0..tcw valid from DRAM
                    nc.sync.dma_start(
                        out=x_t[:, 0 : tcw + 1],
                        in_=bass.AP(
                            xs, xoff + base - 1, [[tcw, NG], [s, D], [1, tcw + 1]]
                        ),
                    )
                    # last group: replicate final sample into the fut halo.
                    # DVE needs a quadrant-aligned partition base, so copy all
                    # of partitions 96..128 first; the halo DMA below then
                    # overwrites 96..120 with the true values.
                    nc.vector.tensor_copy(
                        out=x_t[96:128, tcw + 1 : tcw + 2],
                        in_=x_t[96:128, tcw : tcw + 1],
                    )
                    # fut-halo col tcw+1 for groups 0..14
                    nc.sync.dma_start(
                        out=x_t[0 : 128 - D, tcw + 1 : tcw + 2],
                        in_=bass.AP(
                            xs, xoff + base + tcw, [[tcw, NG - 1], [s, D], [1, 1]]
                        ),
                    )

                # --- load ys supertile (contiguous per group) --------------
                y_t = yp.tile([16, TCG * M], F32R, tag="y")
                nc.sync.dma_start(
                    out=y_t[:, 0 : tcw * M],
                    in_=bass.AP(ys, yoff + base * M, [[tcw * M, NG], [1, tcw * M]]),
                )
                yr = y_t[:, 0 : tcw * M].rearrange("p (t m) -> p m t", m=M)

                if variant == "loads":
                    continue
                o_t = op.tile([128, 3 * TCG], F32, tag="o", name=f"o_{b}_{k}")

                # --- matmuls + PSUM drain, in 512-column halves ------------
                for h0 in range(0, tcw, MW):
                    hw_ = min(MW, tcw - h0)
                    ps = [
                        pp.tile([128, MW], F32, tag=f"p{i}", name=f"p{i}_{b}_{k}_{h0}")
                        for i in range(3)
                    ]
                    cur = x_t[:, 1 + h0 : 1 + h0 + hw_]
                    past = x_t[:, h0 : h0 + hw_]
                    fut = x_t[:, 2 + h0 : 2 + h0 + hw_]
                    p0 = ps[0][:, 0:hw_]
                    p1 = ps[1][:, 0:hw_]
                    p2 = ps[2][:, 0:hw_]
                    nc.tensor.matmul(p0, wr[:, 0:128], cur, start=True, stop=False)
                    nc.tensor.matmul(p0, wr[:, 128:256], past, start=False, stop=True)
                    nc.tensor.matmul(p1, wr[:, 256:384], cur, start=True, stop=False)
                    nc.tensor.matmul(p1, wr[:, 384:512], fut, start=False, stop=True)
                    nc.tensor.matmul(p2, wr[:, 512:640], cur, start=True, stop=False)
                    for m in range(M):
                        c0 = (5 + m) * 128
                        nc.tensor.matmul(
                            p2,
                            wr[0:16, c0 : c0 + 128],
                            yr[:, m, h0 : h0 + hw_],
                            start=False,
                            stop=(m == M - 1),
                        )
                    if variant == "nostores":
                        continue
                    for i in range(3):
                        nc.vector.tensor_copy(
                            out=o_t[:, i * tcw + h0 : i * tcw + h0 + hw_],
                            in_=ps[i][:, 0:hw_],
                        )

                if variant == "nostores":
                    continue
                # --- one merged store DMA per supertile --------------------
                nc.scalar.dma_start(
                    out=bass.AP(
                        m_all, ooff + base, [[tcw, NG], [s, 3 * D], [1, tcw]]
                    ),
                    in_=o_t[:, 0 : 3 * tcw].rearrange("p (o t) -> p o t", o=3),
                )
    nc.finalize()
    return nc


def _build_weights(F, H, Q, R):
    """Host-side precompute (init-time work in the torch module)."""
    F64 = np.asarray(F, np.float64)
    H64 = np.asarray(H, np.float64)
    Q64 = np.asarray(Q, np.float64)
    R64 = np.asarray(R, np.float64)
    Qinv = np.linalg.inv(Q64)
    Rinv = np.linalg.inv(R64)
    A1 = -Qinv
    B1 = Qinv @ F64
    B2 = F64.T @ Qinv
    A2 = -(B2 @ F64)
    C3 = H64.T @ Rinv          # (D, M)
    A3 = -(C3 @ H64)

    w = np.zeros((128, 9 * 128), np.float32)
    eye = np.eye(NG)
    for i, A in enumerate([A1, B1, A2, B2, A3]):
        # lhsT[8g+j, 8g+i] = A[i, j]  ->  block-diag of A.T
        w[:, i * 128 : (i + 1) * 128] = np.kron(eye, A.T).astype(np.float32)
    for m in range(M):
        blk = np.zeros((NG, 128), np.float64)
        for g in range(NG):
            blk[g, D * g : D * g + D] = C3[:, m]
        w[0:NG, (5 + m) * 128 : (6 + m) * 128] = blk.astype(np.float32)
    return w


_CACHE = {}


def _get_nc(bc=BC, s=S):
    key = (bc, s)
    if key not in _CACHE:
        _CACHE[key] = _build_nc(bc, s)
    return _CACHE[key]


def run(xs, ys, F, H, Q, R, trace=False, bc=BC, s=S):
    """Shard across 8 cores, run, gather.  Returns ((m1, m2, m3), results)."""
    xs = np.ascontiguousarray(np.asarray(xs, np.float32))
    ys = np.ascontiguousarray(np.asarray(ys, np.float32))
    w_all = _build_weights(F, H, Q, R)
    nb = xs.shape[0]
    assert nb == bc * N_CORES and xs.shape[1:] == (D, s), xs.shape
    assert ys.shape == (nb, s, M), ys.shape

    nc = _get_nc(bc, s)
    in_maps = [
        {
            "xs": np.ascontiguousarray(xs[i * bc : (i + 1) * bc]),
            "ys": np.ascontiguousarray(ys[i * bc : (i + 1) * bc]),
            "w_all": w_all,
        }
        for i in range(N_CORES)
    ]
    res = run_bass_kernel_spmd(nc, in_maps, core_ids=list(range(N_CORES)), trace=trace)
    m_full = np.concatenate([r["m_all"] for r in res.results], axis=0)  # (B,D,3,s)
    outs = tuple(np.ascontiguousarray(m_full[:, :, i, :]) for i in range(3))
    return outs, res


def kernel(xs, ys, F, H, Q, R):
    trace = bool(int(os.environ.get("KERNEL_TRACE", "0")))
    outs, _ = run(xs, ys, F, H, Q, R, trace=trace)
    return outs



# revision 4
# speedup vs baseline: 2.0888x; 2.0888x over previous
"""Trainium2 Bass kernel for the Kalman graphical-model message-passing problem.

reference math (B=64, D=8, M=4, S=50000):
    m1 = -Qinv @ (xs - F @ x_past)            (B, D, S)
    m2 = FtQinv @ (x_fut - F @ xs)            (B, D, S)
    m3 = HtRinv @ ys_t - (HtRinv @ H) @ xs    (B, D, S)
with x_past/x_fut edge-replicated 1-sample shifts of xs along S.

v2 design (everything bf16 on the wire; rel-err gate is 2e-2, bf16 end-to-end
measures ~7e-3):

  * Algebra: m1 = A1 x_t + B1 x_{t-1} with A1 = -Qinv, B1 = Qinv F.  Then
    m2_t = -F^T m1_{t+1} exactly (including the replicated right edge), so m2
    is ONE matmul over the already-computed m1 tile shifted one column.
    m3 = A3 x_t + C3y y_t with C3 = H^T Rinv, A3 = -(C3 H).
    => 5 matmuls per 512-column chunk instead of 9.

  * Layout: per batch one supertile of NG=16 groups x GW=s/16 samples.
    Partition 8g+j = (group g, state j); columns = samples with one halo
    column each side.  The host PRE-PACKS xs into this exact SBUF image
    (edge replication via clipped gather), and ys transposed into
    partition 4g+m so the ys contraction over m is a single 64-partition
    matmul per chunk (vs 4 stride-4 matmuls).

  * Per batch: 2 load DMAs (x ~800KB, y ~400KB) + 3 store DMAs (~780KB
    each), all with >=6KB contiguous runs.  Per-core traffic ~28.8MB.

  * PSUM fp32, outputs cast to bf16 on the PSUM->SBUF copy (DVE for m1/m2,
    ACT for m3 to split the copy load).
"""

import os
from contextlib import ExitStack

import ml_dtypes
import numpy as np

import concourse.bacc as bacc
import concourse.bass as bass
import concourse.mybir as mybir
import concourse.tile as tile
from concourse.bass_utils import run_bass_kernel_spmd

F32 = mybir.dt.float32
BF16 = mybir.dt.bfloat16
NPBF16 = ml_dtypes.bfloat16

B, D, M, S = 64, 8, 4, 50000
N_CORES = 8
BC = B // N_CORES  # batches per core
NG = 16            # sample groups packed into the 128 partitions
MW = 512           # matmul free-dim / PSUM bank width


def _geom(s):
    assert s % NG == 0, s
    gw = s // NG   # samples per group
    xc = gw + 2    # x cols: 1 past halo + gw + 1 fut halo (cols 0..gw+1 used)
    xc += xc % 2   # pad to even row bytes
    yc = gw + (gw % 2)
    return gw, xc, yc


def _build_nc(bc=BC, s=S):
    variant = os.environ.get("KERNEL_VARIANT", "full")  # perf bisection only
    gw, xc, yc = _geom(s)

    nc = bacc.Bacc(trn_type="TRN2")
    xp = nc.dram_tensor("xp", [bc, 128, xc], BF16, kind="ExternalInput")
    yp = nc.dram_tensor("yp", [bc, 64, yc], BF16, kind="ExternalInput")
    w = nc.dram_tensor("w_all", [128, 5 * 128], BF16, kind="ExternalInput")
    m_all = nc.dram_tensor("m_all", [bc, D, 3, s], BF16, kind="ExternalOutput")

    with tile.TileContext(nc) as tc, ExitStack() as ctx:
        singles = ctx.enter_context(tc.tile_pool(name="singles", bufs=1))
        xpool = ctx.enter_context(tc.tile_pool(name="xp", bufs=3))
        ypool = ctx.enter_context(tc.tile_pool(name="yp", bufs=3))
        o1p = ctx.enter_context(tc.tile_pool(name="o1", bufs=3))
        o2p = ctx.enter_context(tc.tile_pool(name="o2", bufs=3))
        o3p = ctx.enter_context(tc.tile_pool(name="o3", bufs=3))
        pp = ctx.enter_context(tc.tile_pool(name="pp", bufs=2, space="PSUM"))

        w_sb = singles.tile([128, 5 * 128], BF16, tag="w")
        nc.sync.dma_start(out=w_sb[:], in_=w[:, :])
        wA1 = w_sb[:, 0:128]
        wB1 = w_sb[:, 128:256]
        wW2 = w_sb[:, 256:384]
        wA3 = w_sb[:, 384:512]
        wC3 = w_sb[0:64, 512:640]

        for b in range(bc):
            xoff = b * 128 * xc
            yoff = b * 64 * yc
            ooff = b * D * 3 * s

            x_t = xpool.tile([128, xc], BF16, tag="x")
            nc.sync.dma_start(out=x_t[:], in_=bass.AP(xp, xoff, [[xc, 128], [1, xc]]))
            y_t = ypool.tile([64, yc], BF16, tag="y")
            nc.sync.dma_start(out=y_t[:], in_=bass.AP(yp, yoff, [[yc, 64], [1, yc]]))
            if variant == "loads":
                continue

            o1 = o1p.tile([128, gw + 1], BF16, tag="o1", name=f"o1_{b}")
            o2 = o2p.tile([128, gw], BF16, tag="o2", name=f"o2_{b}")
            o3 = o3p.tile([128, gw], BF16, tag="o3", name=f"o3_{b}")

            # m1 over gw+1 cols (one halo col for the m2 shift), m3 over gw
            for h0 in range(0, gw + 1, MW):
                hw1 = min(MW, gw + 1 - h0)
                hw3 = min(MW, gw - h0)
                cur = x_t[:, 1 + h0 : 1 + h0 + hw1]
                past = x_t[:, h0 : h0 + hw1]
                p1 = pp.tile([128, MW], F32, tag="p1", name=f"p1_{b}_{h0}")
                nc.tensor.matmul(p1[:, 0:hw1], wA1, cur, start=True, stop=False)
                nc.tensor.matmul(p1[:, 0:hw1], wB1, past, start=False, stop=True)
                nc.vector.tensor_copy(out=o1[:, h0 : h0 + hw1], in_=p1[:, 0:hw1])
                if hw3 <= 0:
                    continue
                p3 = pp.tile([128, MW], F32, tag="p3", name=f"p3_{b}_{h0}")
                nc.tensor.matmul(
                    p3[:, 0:hw3], wA3, cur[:, 0:hw3], start=True, stop=False
                )
                nc.tensor.matmul(
                    p3[:, 0:hw3], wC3, y_t[:, h0 : h0 + hw3], start=False, stop=True
                )
                nc.scalar.copy(out=o3[:, h0 : h0 + hw3], in_=p3[:, 0:hw3])

            # m2 = -F^T @ m1 shifted one column left
            for h0 in range(0, gw, MW):
                hw = min(MW, gw - h0)
                p2 = pp.tile([128, MW], F32, tag="p2", name=f"p2_{b}_{h0}")
                nc.tensor.matmul(
                    p2[:, 0:hw], wW2, o1[:, 1 + h0 : 1 + h0 + hw], start=True, stop=True
                )
                nc.vector.tensor_copy(out=o2[:, h0 : h0 + hw], in_=p2[:, 0:hw])

            if variant == "nostores":
                continue
            for o_idx, o_t in ((0, o1), (1, o2), (2, o3)):
                nc.scalar.dma_start(
                    out=bass.AP(
                        m_all, ooff + o_idx * s, [[gw, NG], [3 * s, D], [1, gw]]
                    ),
                    in_=o_t[:, 0:gw],
                )
    nc.finalize()
    return nc


def _build_weights(F, H, Q, R):
    """Host-side precompute (init-time work in the torch module)."""
    F64 = np.asarray(F, np.float64)
    H64 = np.asarray(H, np.float64)
    Qinv = np.linalg.inv(np.asarray(Q, np.float64))
    Rinv = np.linalg.inv(np.asarray(R, np.float64))
    A1 = -Qinv
    B1 = Qinv @ F64
    W2 = -F64.T
    C3 = H64.T @ Rinv          # (D, M)
    A3 = -(C3 @ H64)

    eye = np.eye(NG)
    w = np.zeros((128, 5 * 128), NPBF16)
    for i, A in enumerate([A1, B1, W2, A3]):
        # lhsT[8g+j, 8g+i] = A[i, j]  ->  block-diag of A.T
        w[:, i * 128 : (i + 1) * 128] = np.kron(eye, A.T).astype(NPBF16)
    w[0:64, 512:640] = np.kron(eye, C3.T).astype(NPBF16)  # [4g+m, 8g+i] = C3[i, m]
    return w


def _pack_inputs(xs, ys, s):
    """xs (nb, D, s), ys (nb, s, M) f32 -> device images (bf16).

    xp[b, 8g+j, c] = xs[b, j, clip(g*gw + c - 1)]   (c in [0, xc))
    yp[b, 4g+m, c] = ys[b, clip(g*gw + c), m]       (c in [0, yc))
    """
    gw, xc, yc = _geom(s)
    nb = xs.shape[0]
    xs_bf = np.asarray(xs, np.float32).astype(NPBF16)
    g = np.arange(NG)[:, None] * gw
    xcols = np.clip(g + np.arange(xc)[None, :] - 1, 0, s - 1)  # (NG, xc)
    xp = xs_bf[:, :, xcols]                      # (nb, D, NG, xc)
    xp = np.ascontiguousarray(np.swapaxes(xp, 1, 2)).reshape(nb, 128, xc)

    ys_bf = np.swapaxes(np.asarray(ys, np.float32).astype(NPBF16), 1, 2)  # (nb, M, s)
    ycols = np.clip(g + np.arange(yc)[None, :], 0, s - 1)      # (NG, yc)
    yp = ys_bf[:, :, ycols]                      # (nb, M, NG, yc)
    yp = np.ascontiguousarray(np.swapaxes(yp, 1, 2)).reshape(nb, 64, yc)
    return xp, yp


_CACHE = {}


def _get_nc(bc=BC, s=S):
    key = (bc, s)
    if key not in _CACHE:
        _CACHE[key] = _build_nc(bc, s)
    return _CACHE[key]


def run(xs, ys, F, H, Q, R, trace=False, bc=BC, s=S):
    """Shard across 8 cores, run, gather.  Returns ((m1, m2, m3), results)."""
    nb = xs.shape[0]
    assert nb == bc * N_CORES and xs.shape[1:] == (D, s), xs.shape
    assert ys.shape == (nb, s, M), ys.shape
    xp, yp = _pack_inputs(xs, ys, s)
    w_all = _build_weights(F, H, Q, R)

    nc = _get_nc(bc, s)
    in_maps = [
        {
            "xp": np.ascontiguousarray(xp[i * bc : (i + 1) * bc]),
            "yp": np.ascontiguousarray(yp[i * bc : (i + 1) * bc]),
            "w_all": w_all,
        }
        for i in range(N_CORES)
    ]
    res = run_bass_kernel_spmd(nc, in_maps, core_ids=list(range(N_CORES)), trace=trace)
    m_full = np.concatenate([r["m_all"] for r in res.results], axis=0)  # (B,D,3,s) bf16
    outs = tuple(
        np.ascontiguousarray(m_full[:, :, i, :]).astype(np.float32) for i in range(3)
    )
    return outs, res


def kernel(xs, ys, F, H, Q, R):
    trace = bool(int(os.environ.get("KERNEL_TRACE", "0")))
    outs, _ = run(xs, ys, F, H, Q, R, trace=trace)
    return outs


# revision 6
# speedup vs baseline: 2.6286x; 1.2584x over previous
"""Trainium2 Bass kernel for the Kalman graphical-model message-passing problem.

reference math (B=64, D=8, M=4, S=50000):
    m1 = -Qinv @ (xs - F @ x_past)            (B, D, S)
    m2 = FtQinv @ (x_fut - F @ xs)            (B, D, S)
    m3 = HtRinv @ ys_t - (HtRinv @ H) @ xs    (B, D, S)
with x_past/x_fut edge-replicated 1-sample shifts of xs along S.

v2 design (everything bf16 on the wire; rel-err gate is 2e-2, bf16 end-to-end
measures ~7e-3):

  * Algebra: m1 = A1 x_t + B1 x_{t-1} with A1 = -Qinv, B1 = Qinv F.  Then
    m2_t = -F^T m1_{t+1} exactly (including the replicated right edge), so m2
    is ONE matmul over the already-computed m1 tile shifted one column.
    m3 = A3 x_t + C3y y_t with C3 = H^T Rinv, A3 = -(C3 H).
    => 5 matmuls per 512-column chunk instead of 9.

  * Layout: per batch one supertile of NG=16 groups x GW=s/16 samples.
    Partition 8g+j = (group g, state j); columns = samples with one halo
    column each side.  The host PRE-PACKS xs into this exact SBUF image
    (edge replication via clipped gather), and ys transposed into
    partition 4g+m so the ys contraction over m is a single 64-partition
    matmul per chunk (vs 4 stride-4 matmuls).

  * Per batch: 2 load DMAs (x ~800KB, y ~400KB) + 3 store DMAs (~780KB
    each), all with >=6KB contiguous runs.  Per-core traffic ~28.8MB.

  * PSUM fp32, outputs cast to bf16 on the PSUM->SBUF copy (DVE for m1/m2,
    ACT for m3 to split the copy load).
"""

import os
from contextlib import ExitStack

import ml_dtypes
import numpy as np

import concourse.bacc as bacc
import concourse.bass as bass
import concourse.mybir as mybir
import concourse.tile as tile
from concourse.bass_utils import run_bass_kernel_spmd

F32 = mybir.dt.float32
BF16 = mybir.dt.bfloat16
NPBF16 = ml_dtypes.bfloat16

B, D, M, S = 64, 8, 4, 50000
N_CORES = 8
BC = B // N_CORES  # batches per core
NG = 16            # sample groups packed into the 128 partitions
MW = 512           # matmul free-dim / PSUM bank width


def _geom(s):
    assert s % NG == 0, s
    gw = s // NG   # samples per group
    xc = gw + 2    # x cols: 1 past halo + gw + 1 fut halo (cols 0..gw+1 used)
    xc += xc % 2   # pad to even row bytes
    yc = gw + (gw % 2)
    return gw, xc, yc


def _build_nc(bc=BC, s=S):
    variant = os.environ.get("KERNEL_VARIANT", "full")  # perf bisection only
    m2_chain = os.environ.get("KERNEL_M2", "direct") == "chain"
    gw, xc, yc = _geom(s)
    o1w = gw + 1 if m2_chain else gw  # m1 halo col only needed for chaining

    nc = bacc.Bacc(trn_type="TRN2")
    xp = nc.dram_tensor("xp", [bc, 128, xc], BF16, kind="ExternalInput")
    yp = nc.dram_tensor("yp", [bc, 64, yc], BF16, kind="ExternalInput")
    w = nc.dram_tensor("w_all", [128, 7 * 128], BF16, kind="ExternalInput")
    m_all = nc.dram_tensor("m_all", [bc, D, 3, s], BF16, kind="ExternalOutput")

    with tile.TileContext(nc) as tc, ExitStack() as ctx:
        singles = ctx.enter_context(tc.tile_pool(name="singles", bufs=1))
        xpool = ctx.enter_context(tc.tile_pool(name="xp", bufs=3))
        ypool = ctx.enter_context(tc.tile_pool(name="yp", bufs=3))
        o1p = ctx.enter_context(tc.tile_pool(name="o1", bufs=3))
        o2p = ctx.enter_context(tc.tile_pool(name="o2", bufs=3))
        o3p = ctx.enter_context(tc.tile_pool(name="o3", bufs=3))
        pp1 = ctx.enter_context(tc.tile_pool(name="pp1", bufs=3, space="PSUM"))
        pp2 = ctx.enter_context(tc.tile_pool(name="pp2", bufs=2, space="PSUM"))
        pp3 = ctx.enter_context(tc.tile_pool(name="pp3", bufs=3, space="PSUM"))

        w_sb = singles.tile([128, 7 * 128], BF16, tag="w")
        nc.sync.dma_start(out=w_sb[:], in_=w[:, :])
        wA1 = w_sb[:, 0:128]
        wB1 = w_sb[:, 128:256]
        wW2 = w_sb[:, 256:384]
        wA2 = w_sb[:, 384:512]
        wB2 = w_sb[:, 512:640]
        wA3 = w_sb[:, 640:768]
        wC3 = w_sb[0:64, 768:896]

        for b in range(bc):
            xoff = b * 128 * xc
            yoff = b * 64 * yc
            ooff = b * D * 3 * s

            x_t = xpool.tile([128, xc], BF16, tag="x")
            nc.scalar.dma_start(out=x_t[:], in_=bass.AP(xp, xoff, [[xc, 128], [1, xc]]))
            y_t = ypool.tile([64, yc], BF16, tag="y")
            nc.scalar.dma_start(out=y_t[:], in_=bass.AP(yp, yoff, [[yc, 64], [1, yc]]))
            if variant == "loads":
                continue

            o1 = o1p.tile([128, o1w], BF16, tag="o1", name=f"o1_{b}")
            o2 = o2p.tile([128, gw], BF16, tag="o2", name=f"o2_{b}")
            o3 = o3p.tile([128, gw], BF16, tag="o3", name=f"o3_{b}")

            for h0 in range(0, o1w, MW):
                hw1 = min(MW, o1w - h0)
                hw3 = min(MW, gw - h0)
                cur = x_t[:, 1 + h0 : 1 + h0 + hw1]
                past = x_t[:, h0 : h0 + hw1]
                p1 = pp1.tile([128, MW], F32, tag="p1", name=f"p1_{b}_{h0}")
                nc.tensor.matmul(p1[:, 0:hw1], wA1, cur, start=True, stop=False)
                nc.tensor.matmul(p1[:, 0:hw1], wB1, past, start=False, stop=True)
                nc.vector.tensor_copy(out=o1[:, h0 : h0 + hw1], in_=p1[:, 0:hw1])
                if hw3 <= 0:
                    continue
                p3 = pp3.tile([128, MW], F32, tag="p3", name=f"p3_{b}_{h0}")
                nc.tensor.matmul(
                    p3[:, 0:hw3], wA3, cur[:, 0:hw3], start=True, stop=False
                )
                nc.tensor.matmul(
                    p3[:, 0:hw3], wC3, y_t[:, h0 : h0 + hw3], start=False, stop=True
                )
                nc.scalar.copy(out=o3[:, h0 : h0 + hw3], in_=p3[:, 0:hw3])
                if not m2_chain:
                    fut = x_t[:, 2 + h0 : 2 + h0 + hw3]
                    p2 = pp2.tile([128, MW], F32, tag="p2", name=f"p2_{b}_{h0}")
                    nc.tensor.matmul(
                        p2[:, 0:hw3], wA2, cur[:, 0:hw3], start=True, stop=False
                    )
                    nc.tensor.matmul(p2[:, 0:hw3], wB2, fut, start=False, stop=True)
                    nc.vector.tensor_copy(out=o2[:, h0 : h0 + hw3], in_=p2[:, 0:hw3])

            if m2_chain:
                # m2 = -F^T @ m1 shifted one column left
                for h0 in range(0, gw, MW):
                    hw = min(MW, gw - h0)
                    p2 = pp2.tile([128, MW], F32, tag="p2", name=f"p2_{b}_{h0}")
                    nc.tensor.matmul(
                        p2[:, 0:hw],
                        wW2,
                        o1[:, 1 + h0 : 1 + h0 + hw],
                        start=True,
                        stop=True,
                    )
                    nc.vector.tensor_copy(out=o2[:, h0 : h0 + hw], in_=p2[:, 0:hw])

            if variant == "nostores":
                continue
            for o_idx, o_t in ((0, o1), (1, o2), (2, o3)):
                nc.sync.dma_start(
                    out=bass.AP(
                        m_all, ooff + o_idx * s, [[gw, NG], [3 * s, D], [1, gw]]
                    ),
                    in_=o_t[:, 0:gw],
                )
    nc.finalize()
    return nc


def _build_weights(F, H, Q, R):
    """Host-side precompute (init-time work in the torch module)."""
    F64 = np.asarray(F, np.float64)
    H64 = np.asarray(H, np.float64)
    Qinv = np.linalg.inv(np.asarray(Q, np.float64))
    Rinv = np.linalg.inv(np.asarray(R, np.float64))
    A1 = -Qinv
    B1 = Qinv @ F64
    W2 = -F64.T
    C3 = H64.T @ Rinv          # (D, M)
    A3 = -(C3 @ H64)

    A2 = -(F64.T @ Qinv @ F64)
    B2 = F64.T @ Qinv

    eye = np.eye(NG)
    w = np.zeros((128, 7 * 128), NPBF16)
    for i, A in enumerate([A1, B1, W2, A2, B2, A3]):
        # lhsT[8g+j, 8g+i] = A[i, j]  ->  block-diag of A.T
        w[:, i * 128 : (i + 1) * 128] = np.kron(eye, A.T).astype(NPBF16)
    w[0:64, 768:896] = np.kron(eye, C3.T).astype(NPBF16)  # [4g+m, 8g+i] = C3[i, m]
    return w


def _pack_inputs(xs, ys, s):
    """xs (nb, D, s), ys (nb, s, M) f32 -> device images (bf16).

    xp[b, 8g+j, c] = xs[b, j, clip(g*gw + c - 1)]   (c in [0, xc))
    yp[b, 4g+m, c] = ys[b, clip(g*gw + c), m]       (c in [0, yc))
    """
    gw, xc, yc = _geom(s)
    nb = xs.shape[0]
    xs_bf = np.asarray(xs, np.float32).astype(NPBF16)
    g = np.arange(NG)[:, None] * gw
    xcols = np.clip(g + np.arange(xc)[None, :] - 1, 0, s - 1)  # (NG, xc)
    xp = xs_bf[:, :, xcols]                      # (nb, D, NG, xc)
    xp = np.ascontiguousarray(np.swapaxes(xp, 1, 2)).reshape(nb, 128, xc)

    ys_bf = np.swapaxes(np.asarray(ys, np.float32).astype(NPBF16), 1, 2)  # (nb, M, s)
    ycols = np.clip(g + np.arange(yc)[None, :], 0, s - 1)      # (NG, yc)
    yp = ys_bf[:, :, ycols]                      # (nb, M, NG, yc)
    yp = np.ascontiguousarray(np.swapaxes(yp, 1, 2)).reshape(nb, 64, yc)
    return xp, yp


_CACHE = {}


def _get_nc(bc=BC, s=S):
    key = (bc, s)
    if key not in _CACHE:
        _CACHE[key] = _build_nc(bc, s)
    return _CACHE[key]


def run(xs, ys, F, H, Q, R, trace=False, bc=BC, s=S):
    """Shard across 8 cores, run, gather.  Returns ((m1, m2, m3), results)."""
    nb = xs.shape[0]
    assert nb == bc * N_CORES and xs.shape[1:] == (D, s), xs.shape
    assert ys.shape == (nb, s, M), ys.shape
    xp, yp = _pack_inputs(xs, ys, s)
    w_all = _build_weights(F, H, Q, R)

    nc = _get_nc(bc, s)
    in_maps = [
        {
            "xp": np.ascontiguousarray(xp[i * bc : (i + 1) * bc]),
            "yp": np.ascontiguousarray(yp[i * bc : (i + 1) * bc]),
            "w_all": w_all,
        }
        for i in range(N_CORES)
    ]
    res = run_bass_kernel_spmd(nc, in_maps, core_ids=list(range(N_CORES)), trace=trace)
    m_full = np.concatenate([r["m_all"] for r in res.results], axis=0)  # (B,D,3,s) bf16
    outs = tuple(
        np.ascontiguousarray(m_full[:, :, i, :]).astype(np.float32) for i in range(3)
    )
    return outs, res


def kernel(xs, ys, F, H, Q, R):
    trace = bool(int(os.environ.get("KERNEL_TRACE", "0")))
    outs, _ = run(xs, ys, F, H, Q, R, trace=trace)
    return outs
